# revision 27
# baseline (speedup 1.0000x reference)
"""Trainium2 Bass kernel for nn_AttentionModel (seq2seq LSTM with attention).

Sharding: pure data parallelism over batch (256 -> 8 cores x 32), all
weights replicated. Hidden/gate dim lives on SBUF partitions;
(time, batch) on the free axis.

v3 notes (on top of the v2 design below):
- HW-microbenched MM cost on this part: ~25ns fixed + ~0.47ns/moving-col;
  a STRIDED moving operand costs ~+25ns/MM extra.  So `cat` is stored
  slot-major [128, KT, TCAT, B] making every per-slot moving operand
  (whh/lin/topose/scores/catT transposes) contiguous -- worth ~300us.
- All recurrent weights (whh x3, wih x3, lin) are fp8_e3m4 with pow-2
  scales folded into the gate-consuming ACT scale (1/S) and the lin
  copy-back (1/S_lin).  fp8 is NOT faster on the PE (measured) but
  halves SBUF/DRAM footprint, enabling bufs=2 weight pools that
  prefetch the next iteration's weights across the For_i boundary.
- scores use t-major contiguous chunks (16 slots | 5 slots) matching
  PSUM banks; aw/dmask/masked layouts are t-major so A^T is 6 chunk
  transposes (was 21 per-slot transposes).
- topose reads h straight from the decoder cat slot (dec_hs buffer and
  its 25 DVE copies removed).

v2 design notes (vs the original baseline):
- All gate nonlinearities are expressed through Tanh (sigmoid(x) =
  (tanh(x/2)+1)/2, with the 1/2-arg folded into host-scaled weights and
  the residual scales folded into stored-state conventions: h is stored
  as 2h, c as 2c). Decoder softmax uses Exp. Tanh and Exp share one ACT
  LUT table ("exp_and_others"), so the ACT engine never reloads tables
  (1.28us per reload on TRN2).
- The attention context is computed on the PE instead of a serial DVE
  mul+reduce chain: a transposed copy of the attended states catT
  [(slot,batch) rows x H] is maintained incrementally via PE transposes
  + ACT copies, the masked attention weights are transposed into a
  stationary operand, and the context is 12 accumulating matmuls.
  Output transposes back to (h, batch) via PE; all PSUM->SBUF copies of
  this path ride the (otherwise idle) ACT engine.
- Decoder gate accumulation happens directly in one PSUM tile (wih part
  seeds it early, whh part accumulates after attention) - no
  PSUM->SBUF gate staging copies, no identity-fold matmuls.
- Encoder per-step gx fold is 2 N=512 identity matmuls instead of 32
  N=32 ones.

The graded entry point is kernel(**inputs).
"""

import numpy as np
import ml_dtypes

import concourse.bass as bass
import concourse.mybir as mybir
import concourse.tile as tile
from concourse.bass_utils import run_bass_kernel_spmd

BF16 = ml_dtypes.bfloat16
FP8 = ml_dtypes.float8_e3m4
FP32 = mybir.dt.float32
BF = mybir.dt.bfloat16
F8 = mybir.dt.float8e3
F8_MAX = 15.5

N_CORES = 8
B = 32            # batch per core
T_IN = 10
T_OUT = 25
H = 1024
F = 512
P = 66
G = 4 * H         # 4096 gates
KT = H // 128     # 8  k-tiles over hidden
FT = F // 128     # 4  k-tiles over feature
MT = G // 128     # 32 m-tiles over gates
TCAT = 2 * T_IN + 1   # 21 attention slots
SLOT_DEC = 2 * T_IN   # decoder h lives at the LAST slot (20)
NCH = 6               # catT chunks: 5 x 128 rows (4 slots each) + 1 x 32
ACT_TANH = mybir.ActivationFunctionType.Tanh
ACT_EXP = mybir.ActivationFunctionType.Exp
ACT_COPY = mybir.ActivationFunctionType.Copy
ALU_ADD = mybir.AluOpType.add
ALU_MUL = mybir.AluOpType.mult

_MAX_WAITS = 1


def _apply_tile_wait_patches():
    """The walrus CoreV3 codegen in this container rejects instructions
    carrying more than one sync-wait command ("Too many sync wait
    commands"). Keep every instruction at <=1 wait by moving excess waits
    onto same-engine nops emitted immediately before the instruction."""
    import bass_rust
    from concourse.vector_clock import ScopedClock

    SyncInfo = bass_rust.SyncInfo

    def _split_waits(nc, inst):
        si = getattr(inst, "sync_info", None)
        if si is None or not si.on_wait or len(si.on_wait) <= _MAX_WAITS:
            return
        if inst.engine == mybir.EngineType.Unassigned:
            return
        waits = list(si.on_wait)
        si.on_wait = waits[:_MAX_WAITS]
        rest = waits[_MAX_WAITS:]
        eng = nc.engines[inst.engine]
        for i in range(0, len(rest), _MAX_WAITS):
            nop = eng.nop(nofuse=True, hint="wait_split")
            nop.ins.sync_info = SyncInfo(
                on_wait=rest[i:i + _MAX_WAITS], on_update=[]
            )

    orig_commit = tile.TileContext._commit_instruction

    def _commit_split(self, inst, lazy_reg_writes=True):
        si = getattr(inst, "sync_info", None)
        if (si is not None and si.on_wait is not None
                and len(si.on_wait) > _MAX_WAITS
                and inst.engine != mybir.EngineType.Unassigned):
            _split_waits(self.nc, inst)
        return orig_commit(self, inst, lazy_reg_writes)

    tile.TileContext._commit_instruction = _commit_split

    def _drain_and_barrier_split(self, tick_clock, wait_clock):
        drain_inst = self.nc.sync.drain()
        wait_clock.add_sem_waits(
            drain_inst.ins, ScopedClock({None: tick_clock.global_clock})
        )
        sync_info = drain_inst.ins.sync_info
        if sync_info is not None and sync_info.on_wait is not None:
            waits = list(sync_info.on_wait)
            if len(waits) > _MAX_WAITS:
                sync_info.on_wait = waits[:_MAX_WAITS]
                rest = waits[_MAX_WAITS:]
                for i in range(0, len(rest), _MAX_WAITS):
                    nop = self.nc.sync.nop(nofuse=True, hint="drain_wait_split")
                    nop.ins.sync_info = SyncInfo(
                        on_wait=rest[i:i + _MAX_WAITS], on_update=[]
                    )
        self.nc.all_engine_barrier()
        assert self.sems is not None
        popped = self.nc._tile_sem_poison_stack.pop()
        assert popped is self._sem_poison
        self.nc.clear_and_free_semaphores(list(self.sems.allocated().values()))
        self.nc.all_engine_barrier()

    tile.TileContext._drain_and_barrier = _drain_and_barrier_split


_apply_tile_wait_patches()


# ------------------------------------------------------------- host packing

# gate reorder: reference packs gates [i, f, g, o]; we use [i, g, f, o]
# so one Tanh covers (i,g) for the early u-term and one covers (f,o).
_GPERM = np.concatenate([
    np.arange(0, H),              # i
    np.arange(2 * H, 3 * H),      # g
    np.arange(H, 2 * H),          # f
    np.arange(3 * H, 4 * H),      # o
])
# tanh-trick row scale: i,f,o gate rows get 0.5 (tanh of half-arg), g 1.0
_GROWS = np.concatenate([
    np.full(H, 0.5, np.float32), np.ones(H, np.float32),
    np.full(2 * H, 0.5, np.float32)])


def _pack_T(w, ktiles, mcols, dt=BF16):
    """(mcols, ktiles*128) weight -> transposed tiled layout
    (128, ktiles*mcols) with [p, kt*mcols + m] = w[m, kt*128 + p]."""
    if dt is FP8:
        w = np.clip(w, -F8_MAX, F8_MAX)
    wT = np.ascontiguousarray(w.T).astype(dt)      # (ktiles*128, mcols)
    return np.ascontiguousarray(
        wT.reshape(ktiles, 128, mcols).transpose(1, 0, 2)
        .reshape(128, ktiles * mcols))


def _pow2(x):
    return float(2.0 ** round(np.log2(float(x))))


def _prep_weights(inputs):
    """fp8 quantization scheme: the LDWEIGHTS-bound weights (whh x3, dec
    wih, lin) are stored fp8_e3m4 scaled by a power-of-2 S that centers
    their distribution in e3m4's normal range; S is undone in the ACT that
    consumes the accumulated gates (scale=1/S).  The encoder wih stay bf16
    but are host-scaled by the chain's S so gx and whh@h share one scale."""
    d = {}
    scales = {}
    d["tfT"] = np.ascontiguousarray(inputs["tf_w"].T).astype(BF16)  # (66, 512)
    for nm, wih, whh in (("e", "enc_wih", "enc_whh"),
                         ("p", "encp_wih", "encp_whh"),
                         ("d", "dec_wih", "dec_whh")):
        wi = np.asarray(inputs[wih], np.float32)[_GPERM] * _GROWS[:, None]
        # whh consumes stored h'' = 2h -> extra 0.5 on the input side
        wh = (np.asarray(inputs[whh], np.float32)[_GPERM]
              * _GROWS[:, None] * 0.5)
        S = _pow2(0.7 / max(float(wh.std()), 1e-12))
        if nm == "d":
            # dec wih shares S with whh; keep its 4-sigma inside e3m4 range
            while float(wi.std()) * S > F8_MAX / 4.2:
                S /= 2.0
        scales[nm] = S
        d[f"wih_{nm}"] = _pack_T(wi * S, FT, G, dt=FP8)
        d[f"whh_{nm}"] = _pack_T(wh * S, KT, G, dt=FP8)
    # lin/tp consume stored h'' = 2h
    lw = np.asarray(inputs["lin_w"], np.float32) * 0.5
    S_lin = _pow2(1.0 / max(float(lw.std()), 1e-12))
    scales["lin"] = S_lin
    d["linT"] = _pack_T(lw * S_lin, KT, F, dt=FP8)
    d["tpT"] = _pack_T(np.asarray(inputs["tp_w"], np.float32) * 0.5, KT, P)
    d["b_tf"] = np.asarray(inputs["tf_b"], np.float32)
    for nm, bi, bh in (("e", "enc_bih", "enc_bhh"),
                       ("p", "encp_bih", "encp_bhh"),
                       ("d", "dec_bih", "dec_bhh")):
        d[f"b_{nm}"] = ((np.asarray(inputs[bi], np.float32)
                         + np.asarray(inputs[bh], np.float32))[_GPERM]
                        * _GROWS * scales[nm])
    d["b_lin"] = np.asarray(inputs["lin_b"], np.float32) * S_lin
    d["b_tp"] = np.asarray(inputs["tp_b"], np.float32)
    d["scales"] = scales
    return d


def _bias_flags(w):
    return tuple(bool(np.any(w[k])) for k in
                 ("b_tf", "b_e", "b_p", "b_d", "b_lin", "b_tp"))


# ------------------------------------------------------------ device build

def build_model(key=((False,) * 6, 1.0, 1.0, 1.0, 1.0), loop_iters=1,
                ablate=(), warm_fillers=True, debug=False):
    bias_flags, S_e, S_p, S_d, S_lin = key
    gscales = {"e": 1.0 / S_e, "p": 1.0 / S_p, "d": 1.0 / S_d}
    has_btf, has_be, has_bp, has_bd, has_blin, has_btp = bias_flags
    any_bias = any(bias_flags)

    nc = bass.Bass()

    xT_d = nc.dram_tensor("xT", [P, T_IN * B], BF, kind="ExternalInput")
    zT_d = nc.dram_tensor("zT", [P, T_IN * B], BF, kind="ExternalInput")
    residT_d = nc.dram_tensor("residT", [P, T_OUT * B], FP32,
                              kind="ExternalInput")
    tfT_d = nc.dram_tensor("tfT", [P, F], BF, kind="ExternalInput")
    wih_d_d = {}
    whh_d_d = {}
    for nm in ("e", "p", "d"):
        wih_d_d[nm] = nc.dram_tensor(f"wih_{nm}", [128, FT * G], F8,
                                     kind="ExternalInput")
        whh_d_d[nm] = nc.dram_tensor(f"whh_{nm}", [128, KT * G], F8,
                                     kind="ExternalInput")
    linT_d = nc.dram_tensor("linT", [128, KT * F], F8, kind="ExternalInput")
    dmaskT_d = nc.dram_tensor("dmaskT", [B, TCAT * B], BF,
                              kind="ExternalInput")
    tpT_d = nc.dram_tensor("tpT", [128, KT * P], BF, kind="ExternalInput")
    bias_d = {}
    for key, flag, width in (("b_tf", has_btf, F), ("b_e", has_be, G),
                             ("b_p", has_bp, G), ("b_d", has_bd, G),
                             ("b_lin", has_blin, F), ("b_tp", has_btp, P)):
        if flag:
            bias_d[key] = nc.dram_tensor(key, [1, width], BF,
                                         kind="ExternalInput")
    out_d = nc.dram_tensor("oT", [P, T_OUT * B], FP32, kind="ExternalOutput")
    dbg_d = {}
    if debug:
        for nm, shp, dt in (("dbg_xf", [128, FT * T_IN * B], BF),
                            ("dbg_cat", [128, KT * B * TCAT], BF),
                            ("dbg_catT", [128, NCH * H], BF),
                            ("dbg_c", [128, KT * B], FP32),
                            ("dbg_s0", [B, TCAT], FP32),
                            ("dbg_e0", [B, TCAT], BF),
                            ("dbg_aw0", [B, B * TCAT], BF),
                            ("dbg_atth0", [128, KT * B], BF),
                            ("dbg_h1", [128, KT * B], BF),
                            ("dbg_c1", [128, KT * B], FP32),
                            ("dbg_inp0", [128, FT * B], BF),
                            ("dbg_g0", [128, MT * B], FP32),
                            ("dbg_tall0", [128, 4 * KT * B], FP32)):
            dbg_d[nm] = nc.dram_tensor(nm, shp, dt, kind="ExternalOutput")

    with tile.TileContext(nc) as tc:
        with (
            tc.tile_pool(name="singles", bufs=1) as singles,
            tc.tile_pool(name="wih_pool", bufs=2) as wih_pool,
            tc.tile_pool(name="whh_pool", bufs=2) as whh_pool,
            tc.tile_pool(name="gx_pool", bufs=2) as gx_pool,
            tc.tile_pool(name="pG_pool", bufs=2, space="PSUM") as pG_pool,
            tc.tile_pool(name="pShare", bufs=1, space="PSUM") as pShare,
            tc.tile_pool(name="pSmall", bufs=2, space="PSUM") as pSmall,
        ):
            def body(_it=None):
                # ------------- constant/static loads --------------------
                # order/queues chosen so ToFeature inputs land first, then
                # the encoder weights; late-use tensors trail on gpsimd
                tfT = singles.tile([P, F], BF, tag="tfT")
                nc.sync.dma_start(out=tfT, in_=tfT_d[:, :])
                xT = singles.tile([P, T_IN * B], BF, tag="xT")
                nc.sync.dma_start(out=xT, in_=xT_d[:, :])
                zT = singles.tile([P, T_IN * B], BF, tag="zT")
                nc.sync.dma_start(out=zT, in_=zT_d[:, :])
                residT = singles.tile([P, T_OUT * B], FP32, tag="residT")
                nc.sync.dma_start(out=residT, in_=residT_d[:, :])
                linT = singles.tile([128, KT * F], F8, tag="linT")
                nc.sync.dma_start(out=linT, in_=linT_d[:, :])
                tpT = singles.tile([128, KT * P], BF, tag="tpT")
                nc.sync.dma_start(out=tpT, in_=tpT_d[:, :])
                dmaskT = singles.tile([B, TCAT * B], BF, tag="dmaskT")
                nc.sync.dma_start(out=dmaskT, in_=dmaskT_d[:, :])

                bias_sb = {}
                for key, dram in bias_d.items():
                    t = singles.tile(list(dram.shape), BF, tag=key)
                    nc.sync.dma_start(out=t, in_=dram[:, :])
                    bias_sb[key] = t

                ident = singles.tile([128, 128], BF, tag="ident")
                from concourse.masks import make_identity
                make_identity(nc, ident)
                if any_bias:
                    ones_n = singles.tile([1, T_IN * B], BF, tag="ones_n")
                    nc.vector.memset(ones_n, 1.0)

                cat = singles.tile([128, KT, TCAT, B], BF, tag="cat")
                # catT: 6 chunks; chunk j rows r = b*4 + tl represent
                # slot 4j+tl, batch b (chunk 5: rows = batch, slot 20)
                catT = singles.tile([128, NCH, H], BF, tag="catT")
                if debug:
                    nc.vector.memset(catT, 0.0)
                c_e = singles.tile([128, KT, B, 1], FP32, tag="c_e")
                c_p = singles.tile([128, KT, B, 1], FP32, tag="c_p")

                c_bf = singles.tile([128, KT, B, 1], BF, tag="c_bf")
                atth = singles.tile([128, KT, B], BF, tag="atth")
                inp_bf = singles.tile([128, FT, B], BF, tag="inp_bf")
                masked32 = singles.tile([B, B * TCAT], FP32, tag="masked32")
                scoresbt = singles.tile([B, TCAT], FP32, tag="scoresbt")
                neg_mx = singles.tile([B, 1], FP32, tag="neg_mx")
                e_bf = singles.tile([B, TCAT], BF, tag="e_bf")
                e_nrm = singles.tile([B, TCAT], BF, tag="e_nrm")
                ssum32 = singles.tile([B, 1], FP32, tag="ssum32")
                rs32 = singles.tile([B, 1], FP32, tag="rs32")
                aw_m = singles.tile([B, B * TCAT], BF, tag="aw_m")
                A_sb = singles.tile([128, NCH, B], BF, tag="A_sb")
                t_all = singles.tile([128, 4 * KT, B], FP32, tag="t_all")
                th_t = singles.tile([128, KT, B], FP32, tag="th_t")
                u_t = singles.tile([128, KT, B], FP32, tag="u_t")
                v_t = singles.tile([128, KT, B], FP32, tag="v_t")
                oT_sb = singles.tile([P, T_OUT * B], FP32, tag="oT_sb")

                xf = singles.tile([128, FT, T_IN * B], BF, tag="xf")
                zf = singles.tile([128, FT, T_IN * B], BF, tag="zf")

                def pe_filler(dep_ap):
                    # Tiny matmul dependent on a just-produced DVE/ACT tile:
                    # keeps the PE p-state warm across long DVE/ACT chains.
                    if not warm_fillers:
                        return
                    fps = pSmall.tile([1, 8], FP32, tag="sm")
                    nc.tensor.matmul(fps[:, 0:1], dep_ap, dep_ap,
                                     start=True, stop=True)

                # ------------- ToFeature --------------------------------
                def to_feature(src, dst):
                    for ft in range(FT):
                        ps = pG_pool.tile([128, T_IN * B], FP32, tag="pG")
                        nc.tensor.matmul(ps, tfT[:, ft * 128:(ft + 1) * 128],
                                         src[:, :], start=True,
                                         stop=not has_btf)
                        if has_btf:
                            nc.tensor.matmul(
                                ps,
                                bias_sb["b_tf"][0:1, ft * 128:(ft + 1) * 128],
                                ones_n[0:1, :], start=False, stop=True)
                        nc.vector.tensor_copy(out=dst[:, ft, :], in_=ps)

                to_feature(xT, xf)
                to_feature(zT, zf)
                if debug:
                    nc.sync.dma_start(
                        out=dbg_d["dbg_xf"][:, :],
                        in_=xf.rearrange("p a b -> p (a b)"))

                # ------------- encoder gates_x precompute ----------------
                def gates_x(wih_sb, src, dst, bkey, mts=None):
                    for mt in (range(MT) if mts is None else mts):
                        ps = pG_pool.tile([128, T_IN * B], FP32, tag="pG")
                        for kt in range(FT):
                            nc.tensor.matmul(
                                ps,
                                wih_sb[:, kt * G + mt * 128:
                                       kt * G + (mt + 1) * 128],
                                src[:, kt, :],
                                start=(kt == 0),
                                stop=(kt == FT - 1 and bkey is None),
                            )
                        if bkey is not None:
                            nc.tensor.matmul(
                                ps, bias_sb[bkey][0:1, mt * 128:(mt + 1) * 128],
                                ones_n[0:1, :], start=False, stop=True)
                        if mt % 2 == 0:
                            nc.vector.tensor_copy(
                                out=dst[:, :, mt, :],
                                in_=ps.rearrange("p (t b) -> p t b", b=B))
                        else:
                            nc.scalar.copy(
                                out=dst[:, :, mt, :],
                                in_=ps.rearrange("p (t b) -> p t b", b=B))

                wih_e = wih_pool.tile([128, FT * G], F8, tag="wih")
                for kt in range(FT):
                    nc.sync.dma_start(out=wih_e[:, kt * G:(kt + 1) * G],
                                      in_=wih_d_d["e"][:, kt * G:(kt + 1) * G])
                gx_e = gx_pool.tile([128, T_IN, MT, B], BF, tag="gx")
                gates_x(wih_e, xf, gx_e, "b_e" if has_be else None)


                # ------------- LSTM gate tail (tanh-only form) ----------
                # gates packed [i, g, f, o]; t = tanh(gates) (i,f,o at
                # half-arg via host scaling), split in two ACT ops so the
                # DVE chain starts after the first half:
                # u = (t_i+1)*t_g ; v = (t_f+1)*c'' ; c''_new = v/2 + u
                # th = tanh(c''/2) ; h'' = (t_o+1)*th
                def lstm_tail(gsrc, c_tile, h_out, first_step,
                              emit_cbf=False, gscale=1.0):
                    nc.scalar.activation(out=t_all[:, 0:2 * KT, :],
                                         in_=gsrc[:, 0:2 * KT, :],
                                         func=ACT_TANH, scale=gscale)
                    nc.scalar.activation(out=t_all[:, 2 * KT:4 * KT, :],
                                         in_=gsrc[:, 2 * KT:4 * KT, :],
                                         func=ACT_TANH, scale=gscale)
                    cs = c_tile[:, :, :, 0]
                    pe_filler(t_all[0:1, 0:1, 0:1])
                    ti = t_all[:, 0:KT, :]
                    tg = t_all[:, KT:2 * KT, :]
                    tf_ = t_all[:, 2 * KT:3 * KT, :]
                    to = t_all[:, 3 * KT:4 * KT, :]
                    if first_step:
                        nc.vector.scalar_tensor_tensor(
                            out=cs, in0=ti, scalar=1.0, in1=tg,
                            op0=ALU_ADD, op1=ALU_MUL)
                    else:
                        nc.vector.scalar_tensor_tensor(
                            out=u_t, in0=ti, scalar=1.0, in1=tg,
                            op0=ALU_ADD, op1=ALU_MUL)
                        nc.vector.scalar_tensor_tensor(
                            out=v_t, in0=tf_, scalar=1.0, in1=cs,
                            op0=ALU_ADD, op1=ALU_MUL)
                        nc.vector.scalar_tensor_tensor(
                            out=cs, in0=v_t, scalar=0.5, in1=u_t,
                            op0=ALU_MUL, op1=ALU_ADD)
                    if emit_cbf:
                        # scores want true c = c''/2, against cat'' = 2h:
                        # c_bf = c''/4
                        nc.vector.tensor_scalar_mul(c_bf[:, :, :, 0], cs, 0.25)
                    # th and h split by kt-halves: kt 0:4 consumers of the
                    # new h (whh/scoresB/catT/lin kt loops) start one half-op
                    # earlier; the full-width ops would gate them on all of h
                    HK = KT // 2
                    nc.scalar.activation(out=th_t[:, 0:HK, :],
                                         in_=cs[:, 0:HK, :], func=ACT_TANH,
                                         scale=0.5)
                    pe_filler(th_t[0:1, 0:1, 0:1])
                    nc.vector.scalar_tensor_tensor(
                        out=h_out[:, 0:HK, :], in0=to[:, 0:HK, :], scalar=1.0,
                        in1=th_t[:, 0:HK, :],
                        op0=ALU_ADD, op1=ALU_MUL)
                    nc.scalar.activation(out=th_t[:, HK:KT, :],
                                         in_=cs[:, HK:KT, :], func=ACT_TANH,
                                         scale=0.5)
                    nc.vector.scalar_tensor_tensor(
                        out=h_out[:, HK:KT, :], in0=to[:, HK:KT, :],
                        scalar=1.0, in1=th_t[:, HK:KT, :],
                        op0=ALU_ADD, op1=ALU_MUL)

                # ------------- catT chunk maintenance --------------------
                # chunk j rows r = tl*32 + b represent slot 4j+tl, batch b
                chunk_state = {}

                def catT_slot(slot):
                    j, tl = divmod(slot, 4)
                    if tl == 0:
                        chunk_state[j] = pShare.tile([128, H], BF, tag="pA",
                                                     name=f"tch{j}")
                    tps = chunk_state[j]
                    for kt in range(KT):
                        nc.tensor.matmul(
                            tps[tl * 32:(tl + 1) * 32,
                                kt * 128:(kt + 1) * 128],
                            cat[:, kt, slot, :],
                            ident, is_transpose=True, start=True, stop=True,
                            tile_position=(0, tl * 32),
                            skip_group_check=(tl > 0))
                    if tl == 3:
                        nc.scalar.copy(out=catT[:, j, :], in_=tps)
                        del chunk_state[j]

                def catT_dec_slot():
                    # decoder slot 20 -> catT chunk 5 rows 0..31; copy-back
                    # split across ACT and DVE
                    tps = pSmall.tile([B, H], BF, tag="sm")
                    for kt in range(KT):
                        nc.tensor.matmul(
                            tps[:, kt * 128:(kt + 1) * 128],
                            cat[:, kt, SLOT_DEC, :],
                            ident, is_transpose=True, start=True, stop=True)
                    nc.scalar.copy(out=catT[0:B, NCH - 1, 0:512],
                                   in_=tps[:, 0:512])
                    nc.vector.tensor_copy(out=catT[0:B, NCH - 1, 512:H],
                                          in_=tps[:, 512:H])

                # ------------- encoder chains ----------------------------
                def run_encoder(whh_sb, gx, c_tile, slot0, gscale,
                                fill=None):
                    for t in range(T_IN):
                        slot = slot0 + t
                        h_out = cat[:, :, slot, :]
                        if t == 0:
                            lstm_tail(gx[:, 0, :, :], c_tile, h_out, True,
                                      gscale=gscale)
                            if fill is not None:
                                fill(t)
                        else:
                            gps = pG_pool.tile([128, MT, B], FP32, tag="pG")
                            # seed with gx[t] via two N=512 identity folds
                            for hh in range(2):
                                nc.tensor.matmul(
                                    gps[:, hh * 16:(hh + 1) * 16, :].rearrange(
                                        "p m b -> p (m b)"),
                                    ident,
                                    gx[:, t, hh * 16:(hh + 1) * 16, :]
                                    .rearrange("p m b -> p (m b)"),
                                    start=True, stop=False)
                            prev = cat[:, :, slot - 1, :]
                            for kt in range(KT):
                                for mt in range(MT):
                                    nc.tensor.matmul(
                                        gps[:, mt, :],
                                        whh_sb[:, kt * G + mt * 128:
                                               kt * G + (mt + 1) * 128],
                                        prev[:, kt, :],
                                        start=False,
                                        stop=(kt == KT - 1
                                              and mt % 16 == 15),
                                    )
                            if fill is not None:
                                fill(t)
                            lstm_tail(gps, c_tile, h_out, False,
                                      gscale=gscale)
                        catT_slot(slot)

                whh_e = whh_pool.tile([128, KT * G], F8, tag="whh")
                for kt in range(KT):
                    nc.sync.dma_start(out=whh_e[:, kt * G:(kt + 1) * G],
                                      in_=whh_d_d["e"][:, kt * G:(kt + 1) * G])
                wih_p = wih_pool.tile([128, FT * G], F8, tag="wih")
                for kt in range(FT):
                    nc.sync.dma_start(out=wih_p[:, kt * G:(kt + 1) * G],
                                      in_=wih_d_d["p"][:, kt * G:(kt + 1) * G])
                gx_p = gx_pool.tile([128, T_IN, MT, B], BF, tag="gx")
                if "enc" not in ablate:
                    bkey_p = "b_p" if has_bp else None

                    def _fill_e(t):
                        gates_x(wih_p, zf, gx_p, bkey_p,
                                mts=(2 * t, 2 * t + 1))

                    run_encoder(whh_e, gx_e, c_e, 0, gscales["e"],
                                fill=_fill_e)
                    gates_x(wih_p, zf, gx_p, bkey_p,
                            mts=range(2 * T_IN, MT))
                else:
                    gates_x(wih_p, zf, gx_p, "b_p" if has_bp else None)
                    nc.vector.memset(cat, 0.01)
                    nc.vector.memset(catT, 0.01)
                    nc.vector.memset(c_e, 0.01)
                    nc.vector.memset(c_p, 0.01)

                whh_p = whh_pool.tile([128, KT * G], F8, tag="whh")
                for kt in range(KT):
                    nc.sync.dma_start(out=whh_p[:, kt * G:(kt + 1) * G],
                                      in_=whh_d_d["p"][:, kt * G:(kt + 1) * G])
                if "enc" not in ablate:
                    run_encoder(whh_p, gx_p, c_p, T_IN, gscales["p"])

                # ------------- decoder ----------------------------------
                wih_dd = wih_pool.tile([128, FT * G], F8, tag="wih")
                for kt in range(FT):
                    nc.sync.dma_start(out=wih_dd[:, kt * G:(kt + 1) * G],
                                      in_=wih_d_d["d"][:, kt * G:(kt + 1) * G])
                whh_dd = whh_pool.tile([128, KT * G], F8, tag="whh")
                for kt in range(KT):
                    nc.sync.dma_start(out=whh_dd[:, kt * G:(kt + 1) * G],
                                      in_=whh_d_d["d"][:, kt * G:(kt + 1) * G])

                nc.vector.tensor_copy(out=cat[:, :, SLOT_DEC, :],
                                      in_=cat[:, :, T_IN - 1, :])
                c_d = c_e
                nc.vector.tensor_scalar_mul(c_bf[:, :, :, 0],
                                            c_d[:, :, :, 0], 0.25)

                def topose_mm(ti_):
                    # out column block for one decoder step; runs in the PE
                    # bubble while the current step's DVE/ACT tail executes.
                    # Reads the h still sitting in the decoder cat slot.
                    tps_o = pSmall.tile([P, B], FP32, tag="sm")
                    for kt in range(KT):
                        nc.tensor.matmul(
                            tps_o,
                            tpT[:, kt * P:(kt + 1) * P],
                            cat[:, kt, SLOT_DEC, :],
                            start=(kt == 0),
                            stop=(kt == KT - 1 and not has_btp))
                    if has_btp:
                        nc.tensor.matmul(
                            tps_o, bias_sb["b_tp"][0:1, :],
                            ones_n[0:1, 0:B], start=False, stop=True)
                    return tps_o

                def topose_add(ti_, tps_o):
                    nc.vector.tensor_add(
                        oT_sb[:, ti_ * B:(ti_ + 1) * B],
                        tps_o,
                        residT[:, ti_ * B:(ti_ + 1) * B])
                if "dec" not in ablate:
                    catT_dec_slot()

                if debug:
                    nc.sync.dma_start(
                        out=dbg_d["dbg_cat"][:, :],
                        in_=cat.rearrange("p a b c -> p (a b c)"))
                    nc.sync.dma_start(
                        out=dbg_d["dbg_catT"][:, :],
                        in_=catT.rearrange("p a b -> p (a b)"))
                    nc.sync.dma_start(
                        out=dbg_d["dbg_c"][:, :],
                        in_=c_d.rearrange("p a b c -> p (a b c)"))

                nhalf = B // 2

                dec_steps = 0 if "dec" in ablate else T_OUT
                for t in range(dec_steps):
                    # --- scores: diagonal matmul, c_bf stationary;
                    # t-major contiguous moving chunks (16 slots | 5 slots).
                    # All chunk-A MMs first: they read only static encoder
                    # slots + c_bf (ready mid-tail), while chunk B includes
                    # slot 20 (the fresh h, ready only at tail end) -- the
                    # FIFO PE would otherwise stall chunk A behind B ---
                    scd = pShare.tile([B, 2, 512], FP32, tag="pA")
                    for kt in range(KT):
                        nc.tensor.matmul(
                            scd[:, 0, 0:16 * B],
                            c_bf[:, kt, :, 0],
                            cat[:, kt, 0:16, :].rearrange(
                                "p t b -> p (t b)"),
                            start=(kt == 0), stop=(kt == KT - 1))
                    for kt in range(KT):
                        nc.tensor.matmul(
                            scd[:, 1, 0:(TCAT - 16) * B],
                            c_bf[:, kt, :, 0],
                            cat[:, kt, 16:TCAT, :].rearrange(
                                "p t b -> p (t b)"),
                            start=(kt == 0), stop=(kt == KT - 1))

                    # catT chunk 5 for THIS step's attention: transposes
                    # the previous h (cat slot untouched since the last
                    # tail); emitted after scores so the PE isn't FIFO-
                    # blocked behind its full-tail dependency
                    if t > 0:
                        catT_dec_slot()

                    # --- lin: inp = h'' @ (lin_w/2).T  (on PE) ---
                    ips = pSmall.tile([128, FT, B], FP32, tag="sm")
                    for kt in range(KT):
                        for mt in range(FT):
                            nc.tensor.matmul(
                                ips[:, mt, :],
                                linT[:, kt * F + mt * 128:
                                     kt * F + (mt + 1) * 128],
                                cat[:, kt, SLOT_DEC, :],
                                start=(kt == 0 and mt == 0),
                                stop=(kt == KT - 1 and not has_blin
                                      and mt == FT - 1),
                            )
                    if has_blin:
                        for mt in range(FT):
                            nc.tensor.matmul(
                                ips[:, mt, :],
                                bias_sb["b_lin"][0:1, mt * 128:(mt + 1) * 128],
                                ones_n[0:1, 0:B], start=False,
                                stop=(mt == FT - 1))
                    nc.scalar.activation(out=inp_bf, in_=ips,
                                         func=ACT_COPY,
                                         scale=1.0 / S_lin)
                    if debug and t == 0:
                        nc.sync.dma_start(
                            out=dbg_d["dbg_inp0"][:, :],
                            in_=inp_bf.rearrange("p a b -> p (a b)"))

                    # --- wih gates seed the PSUM accumulator early ---
                    gps = pG_pool.tile([128, MT, B], FP32, tag="pG")
                    # start only on the first write of each PSUM bank: the
                    # bank-wide pending-zero makes every later first-touch
                    # store, and accumulation chains stay intact
                    for kt in range(FT):
                        for mt in range(MT):
                            nc.tensor.matmul(
                                gps[:, mt, :],
                                wih_dd[:, kt * G + mt * 128:
                                       kt * G + (mt + 1) * 128],
                                inp_bf[:, kt, :],
                                start=(kt == 0 and mt % 16 == 0),
                                stop=False)
                    if has_bd:
                        for mt in range(MT):
                            nc.tensor.matmul(
                                gps[:, mt, :],
                                bias_sb["b_d"][0:1, mt * 128:(mt + 1) * 128],
                                ones_n[0:1, 0:B], start=False, stop=False)

                    # --- softmax over 21 slots (batch on partitions) ---
                    nc.vector.tensor_mul(
                        masked32[:, 0:16 * B],
                        scd[:, 0, 0:16 * B],
                        dmaskT[:, 0:16 * B])
                    nc.vector.tensor_mul(
                        masked32[:, 16 * B:TCAT * B],
                        scd[:, 1, 0:(TCAT - 16) * B],
                        dmaskT[:, 16 * B:TCAT * B])
                    nc.vector.tensor_reduce(
                        scoresbt,
                        masked32.rearrange("p (t b) -> p t b", t=TCAT),
                        axis=mybir.AxisListType.X, op=mybir.AluOpType.add)
                    pe_filler(scoresbt[:, 0:1])
                    nc.vector.tensor_reduce(
                        neg_mx, scoresbt, axis=mybir.AxisListType.X,
                        op=mybir.AluOpType.max, negate=True)
                    nc.scalar.activation(
                        out=e_bf, in_=scoresbt, func=ACT_EXP,
                        bias=neg_mx, accum_out=ssum32)
                    nc.vector.reciprocal(rs32, ssum32)
                    # normalized weights: 1/sum folded here so the direct
                    # (h,b)-layout context matmuls need no output scaling
                    nc.vector.tensor_mul(
                        e_nrm, e_bf, rs32.to_broadcast((B, TCAT)))
                    nc.vector.tensor_mul(
                        aw_m.rearrange("p (t b) -> p t b", t=TCAT),
                        dmaskT.rearrange("p (t b) -> p t b", t=TCAT),
                        e_nrm.unsqueeze(2).to_broadcast((B, TCAT, B)))
                    if debug and t == 0:
                        nc.sync.dma_start(out=dbg_d["dbg_s0"][:, :],
                                          in_=scoresbt)
                        nc.sync.dma_start(out=dbg_d["dbg_e0"][:, :], in_=e_bf)
                        nc.sync.dma_start(out=dbg_d["dbg_aw0"][:, :], in_=aw_m)

                    # --- context via PE: A = aw_m^T in 6 chunk
                    #     transposes, then ctxT(b,h) = sum_j A_j^T @ catT_j ---
                    pA = pSmall.tile([128, NCH, B], BF, tag="sm")
                    for j in range(NCH):
                        cols = 128 if j < NCH - 1 else B
                        nc.tensor.matmul(
                            pA[0:cols, j, :],
                            aw_m[:, j * 128:j * 128 + cols],
                            ident[0:B, 0:B],
                            is_transpose=True, start=True, stop=True)
                    nc.scalar.copy(out=A_sb[:, 0:NCH - 1, :],
                                   in_=pA[:, 0:NCH - 1, :])
                    nc.vector.tensor_copy(out=A_sb[0:B, NCH - 1, :],
                                          in_=pA[0:B, NCH - 1, :])

                    # context computed directly in (h, b) layout:
                    # atth(h,b) = sum_j catT_j[:, hslice].T @ A_j ; per
                    # h-half copies (ACT then DVE) so whh chases half 0
                    atps = pSmall.tile([128, KT, B], FP32, tag="sm")
                    for hh in range(2):
                        for kt in range(4 * hh, 4 * hh + 4):
                            for j in range(NCH):
                                rows = 128 if j < NCH - 1 else B
                                nc.tensor.matmul(
                                    atps[:, kt, :],
                                    catT[0:rows, j,
                                         kt * 128:(kt + 1) * 128],
                                    A_sb[0:rows, j, :],
                                    start=(j == 0), stop=(j == NCH - 1))
                        if hh == 0:
                            nc.scalar.copy(
                                out=atth[:, 0:4, :], in_=atps[:, 0:4, :])
                        else:
                            nc.vector.tensor_copy(
                                out=atth[:, 4:8, :], in_=atps[:, 4:8, :])
                    if debug and t == 0:
                        nc.sync.dma_start(
                            out=dbg_d["dbg_atth0"][:, :],
                            in_=atth.rearrange("p a b -> p (a b)"))

                    # --- whh gates accumulate onto the wih seed ---
                    for hf in range(2):
                        for kt in range(4 * hf, 4 * hf + 4):
                            for mt in range(MT):
                                nc.tensor.matmul(
                                    gps[:, mt, :],
                                    whh_dd[:, kt * G + mt * 128:
                                           kt * G + (mt + 1) * 128],
                                    atth[:, kt, :],
                                    start=False,
                                    stop=(kt == KT - 1 and mt % 16 == 15))

                    if debug and t == 0:
                        dbg_g = gx_pool.tile([128, MT, B], FP32, tag="dbgg",
                                             bufs=1)
                        nc.vector.tensor_copy(out=dbg_g, in_=gps)
                        nc.sync.dma_start(
                            out=dbg_d["dbg_g0"][:, :],
                            in_=dbg_g.rearrange("p a b -> p (a b)"))
                    tp_ps = topose_mm(t - 1) if t > 0 else None
                    lstm_tail(gps, c_d, cat[:, :, SLOT_DEC, :], False,
                              emit_cbf=(t < T_OUT - 1),
                              gscale=gscales["d"])
                    if debug and t == 0:
                        nc.sync.dma_start(
                            out=dbg_d["dbg_tall0"][:, :],
                            in_=t_all.rearrange("p a b -> p (a b)"))
                    if tp_ps is not None:
                        topose_add(t - 1, tp_ps)
                    if debug and t == 0:
                        dbgh1 = gx_pool.tile([128, KT, B], BF, tag="dbgh1",
                                             bufs=1)
                        nc.vector.tensor_copy(out=dbgh1,
                                              in_=cat[:, :, SLOT_DEC, :])
                        nc.sync.dma_start(
                            out=dbg_d["dbg_h1"][:, :],
                            in_=dbgh1.rearrange("p a b -> p (a b)"))
                        nc.sync.dma_start(
                            out=dbg_d["dbg_c1"][:, :],
                            in_=c_d.rearrange("p a b c -> p (a b c)"))
                # ------------- ToPose tail + output ---------------------
                if "dec" in ablate:
                    return
                topose_add(T_OUT - 1, topose_mm(T_OUT - 1))
                nc.sync.dma_start(out=out_d[:, :], in_=oT_sb)

            if loop_iters > 1:
                with tc.For_i(0, loop_iters, 1, name="rep"):
                    body()
            else:
                body()

    return nc


# ------------------------------------------------------------- entry point

_model_cache = {}


def _get_model(key):
    if key not in _model_cache:
        _model_cache[key] = build_model(key)
    return _model_cache[key]


def make_in_maps(inputs):
    """Host-side packing: returns per-core input maps and the model key."""
    w = _prep_weights(inputs)
    sc = w.pop("scales")
    flags = _bias_flags(w)
    key = (flags, sc["e"], sc["p"], sc["d"], sc["lin"])
    x = np.asarray(inputs["x"], dtype=np.float32)
    z = np.asarray(inputs["z"], dtype=np.float32)
    fr = np.asarray(inputs["for_resid"], dtype=np.float32)

    dmask = np.zeros((B, B, TCAT), dtype=np.float32)
    for b in range(B):
        dmask[b, b, :] = 1.0
    shared = {
        "tfT": w["tfT"], "linT": w["linT"], "tpT": w["tpT"],
        "dmaskT": np.ascontiguousarray(
            dmask.transpose(0, 2, 1).reshape(B, TCAT * B)).astype(BF16),
    }
    for nm in ("e", "p", "d"):
        shared[f"wih_{nm}"] = w[f"wih_{nm}"]
        shared[f"whh_{nm}"] = w[f"whh_{nm}"]
    names = ("b_tf", "b_e", "b_p", "b_d", "b_lin", "b_tp")
    for f, name in zip(flags, names):
        if f:
            shared[name] = np.ascontiguousarray(
                w[name][None, :]).astype(BF16)

    in_maps = []
    for c in range(N_CORES):
        sl = slice(c * B, (c + 1) * B)
        m = dict(shared)
        m["xT"] = np.ascontiguousarray(
            x[sl].transpose(2, 1, 0).reshape(P, T_IN * B)).astype(BF16)
        m["zT"] = np.ascontiguousarray(
            z[sl].transpose(2, 1, 0).reshape(P, T_IN * B)).astype(BF16)
        m["residT"] = np.ascontiguousarray(
            fr[sl].transpose(2, 1, 0).reshape(P, T_OUT * B))
        in_maps.append(m)
    return in_maps, key


def unshard_output(results):
    outs = []
    for c in range(N_CORES):
        oT = np.asarray(results[c]["oT"])  # (66, 800)
        outs.append(oT.reshape(P, T_OUT, B).transpose(2, 1, 0))
    return np.ascontiguousarray(np.concatenate(outs, axis=0),
                                dtype=np.float32)


def kernel(**inputs) -> np.ndarray:
    in_maps, key = make_in_maps(inputs)
    nc = _get_model(key)
    res = run_bass_kernel_spmd(nc, in_maps, core_ids=list(range(N_CORES)))
    return unshard_output(res.results)



# revision 28
# speedup vs baseline: 1.0610x; 1.0610x over previous
"""Trainium2 Bass kernel for nn_AttentionModel (seq2seq LSTM with attention).

Sharding: pure data parallelism over batch (256 -> 8 cores x 32), all
weights replicated. Hidden/gate dim lives on SBUF partitions;
(time, batch) on the free axis.

v3 notes (on top of the v2 design below):
- HW-microbenched MM cost on this part: ~25ns fixed + ~0.47ns/moving-col;
  a STRIDED moving operand costs ~+25ns/MM extra.  So `cat` is stored
  slot-major [128, KT, TCAT, B] making every per-slot moving operand
  (whh/lin/topose/scores/catT transposes) contiguous -- worth ~300us.
- All recurrent weights (whh x3, wih x3, lin) are fp8_e3m4 with pow-2
  scales folded into the gate-consuming ACT scale (1/S) and the lin
  copy-back (1/S_lin).  fp8 is NOT faster on the PE (measured) but
  halves SBUF/DRAM footprint, enabling bufs=2 weight pools that
  prefetch the next iteration's weights across the For_i boundary.
- scores use t-major contiguous chunks (16 slots | 5 slots) matching
  PSUM banks; aw/dmask/masked layouts are t-major so A^T is 6 chunk
  transposes (was 21 per-slot transposes).
- topose reads h straight from the decoder cat slot (dec_hs buffer and
  its 25 DVE copies removed).

v2 design notes (vs the original baseline):
- All gate nonlinearities are expressed through Tanh (sigmoid(x) =
  (tanh(x/2)+1)/2, with the 1/2-arg folded into host-scaled weights and
  the residual scales folded into stored-state conventions: h is stored
  as 2h, c as 2c). Decoder softmax uses Exp. Tanh and Exp share one ACT
  LUT table ("exp_and_others"), so the ACT engine never reloads tables
  (1.28us per reload on TRN2).
- The attention context is computed on the PE instead of a serial DVE
  mul+reduce chain: a transposed copy of the attended states catT
  [(slot,batch) rows x H] is maintained incrementally via PE transposes
  + ACT copies, the masked attention weights are transposed into a
  stationary operand, and the context is 12 accumulating matmuls.
  Output transposes back to (h, batch) via PE; all PSUM->SBUF copies of
  this path ride the (otherwise idle) ACT engine.
- Decoder gate accumulation happens directly in one PSUM tile (wih part
  seeds it early, whh part accumulates after attention) - no
  PSUM->SBUF gate staging copies, no identity-fold matmuls.
- Encoder per-step gx fold is 2 N=512 identity matmuls instead of 32
  N=32 ones.

The graded entry point is kernel(**inputs).
"""

import numpy as np
import ml_dtypes

import concourse.bass as bass
import concourse.mybir as mybir
import concourse.tile as tile
from concourse.bass_utils import run_bass_kernel_spmd

BF16 = ml_dtypes.bfloat16
FP8 = ml_dtypes.float8_e3m4
FP32 = mybir.dt.float32
BF = mybir.dt.bfloat16
F8 = mybir.dt.float8e3
F8_MAX = 15.5

N_CORES = 8
B = 32            # batch per core
T_IN = 10
T_OUT = 25
H = 1024
F = 512
P = 66
G = 4 * H         # 4096 gates
KT = H // 128     # 8  k-tiles over hidden
FT = F // 128     # 4  k-tiles over feature
MT = G // 128     # 32 m-tiles over gates
TCAT = 2 * T_IN + 1   # 21 attention slots
SLOT_DEC = 2 * T_IN   # decoder h lives at the LAST slot (20)
NCH = 6               # catT chunks: 5 x 128 rows (4 slots each) + 1 x 32
ACT_TANH = mybir.ActivationFunctionType.Tanh
ACT_EXP = mybir.ActivationFunctionType.Exp
ACT_COPY = mybir.ActivationFunctionType.Copy
ALU_ADD = mybir.AluOpType.add
ALU_MUL = mybir.AluOpType.mult

_MAX_WAITS = 1


def _apply_tile_wait_patches():
    """The walrus CoreV3 codegen in this container rejects instructions
    carrying more than one sync-wait command ("Too many sync wait
    commands"). Keep every instruction at <=1 wait by moving excess waits
    onto same-engine nops emitted immediately before the instruction."""
    import bass_rust
    from concourse.vector_clock import ScopedClock

    SyncInfo = bass_rust.SyncInfo

    def _split_waits(nc, inst):
        si = getattr(inst, "sync_info", None)
        if si is None or not si.on_wait or len(si.on_wait) <= _MAX_WAITS:
            return
        if inst.engine == mybir.EngineType.Unassigned:
            return
        waits = list(si.on_wait)
        si.on_wait = waits[:_MAX_WAITS]
        rest = waits[_MAX_WAITS:]
        eng = nc.engines[inst.engine]
        for i in range(0, len(rest), _MAX_WAITS):
            nop = eng.nop(nofuse=True, hint="wait_split")
            nop.ins.sync_info = SyncInfo(
                on_wait=rest[i:i + _MAX_WAITS], on_update=[]
            )

    orig_commit = tile.TileContext._commit_instruction

    def _commit_split(self, inst, lazy_reg_writes=True):
        si = getattr(inst, "sync_info", None)
        if (si is not None and si.on_wait is not None
                and len(si.on_wait) > _MAX_WAITS
                and inst.engine != mybir.EngineType.Unassigned):
            _split_waits(self.nc, inst)
        return orig_commit(self, inst, lazy_reg_writes)

    tile.TileContext._commit_instruction = _commit_split

    def _drain_and_barrier_split(self, tick_clock, wait_clock):
        drain_inst = self.nc.sync.drain()
        wait_clock.add_sem_waits(
            drain_inst.ins, ScopedClock({None: tick_clock.global_clock})
        )
        sync_info = drain_inst.ins.sync_info
        if sync_info is not None and sync_info.on_wait is not None:
            waits = list(sync_info.on_wait)
            if len(waits) > _MAX_WAITS:
                sync_info.on_wait = waits[:_MAX_WAITS]
                rest = waits[_MAX_WAITS:]
                for i in range(0, len(rest), _MAX_WAITS):
                    nop = self.nc.sync.nop(nofuse=True, hint="drain_wait_split")
                    nop.ins.sync_info = SyncInfo(
                        on_wait=rest[i:i + _MAX_WAITS], on_update=[]
                    )
        self.nc.all_engine_barrier()
        assert self.sems is not None
        popped = self.nc._tile_sem_poison_stack.pop()
        assert popped is self._sem_poison
        self.nc.clear_and_free_semaphores(list(self.sems.allocated().values()))
        self.nc.all_engine_barrier()

    tile.TileContext._drain_and_barrier = _drain_and_barrier_split


_apply_tile_wait_patches()


# ------------------------------------------------------------- host packing

# gate reorder: reference packs gates [i, f, g, o]; we use [i, g, f, o]
# so one Tanh covers (i,g) for the early u-term and one covers (f,o).
_GPERM = np.concatenate([
    np.arange(0, H),              # i
    np.arange(2 * H, 3 * H),      # g
    np.arange(H, 2 * H),          # f
    np.arange(3 * H, 4 * H),      # o
])
# tanh-trick row scale: i,f,o gate rows get 0.5 (tanh of half-arg), g 1.0
_GROWS = np.concatenate([
    np.full(H, 0.5, np.float32), np.ones(H, np.float32),
    np.full(2 * H, 0.5, np.float32)])


def _pack_T(w, ktiles, mcols, dt=BF16):
    """(mcols, ktiles*128) weight -> transposed tiled layout
    (128, ktiles*mcols) with [p, kt*mcols + m] = w[m, kt*128 + p]."""
    if dt is FP8:
        w = np.clip(w, -F8_MAX, F8_MAX)
    wT = np.ascontiguousarray(w.T).astype(dt)      # (ktiles*128, mcols)
    return np.ascontiguousarray(
        wT.reshape(ktiles, 128, mcols).transpose(1, 0, 2)
        .reshape(128, ktiles * mcols))


def _pow2(x):
    return float(2.0 ** round(np.log2(float(x))))


def _prep_weights(inputs):
    """fp8 quantization scheme: the LDWEIGHTS-bound weights (whh x3, dec
    wih, lin) are stored fp8_e3m4 scaled by a power-of-2 S that centers
    their distribution in e3m4's normal range; S is undone in the ACT that
    consumes the accumulated gates (scale=1/S).  The encoder wih stay bf16
    but are host-scaled by the chain's S so gx and whh@h share one scale."""
    d = {}
    scales = {}
    d["tfT"] = np.ascontiguousarray(inputs["tf_w"].T).astype(BF16)  # (66, 512)
    for nm, wih, whh in (("e", "enc_wih", "enc_whh"),
                         ("p", "encp_wih", "encp_whh"),
                         ("d", "dec_wih", "dec_whh")):
        wi = np.asarray(inputs[wih], np.float32)[_GPERM] * _GROWS[:, None]
        # whh consumes stored h'' = 2h -> extra 0.5 on the input side
        wh = (np.asarray(inputs[whh], np.float32)[_GPERM]
              * _GROWS[:, None] * 0.5)
        S = _pow2(0.7 / max(float(wh.std()), 1e-12))
        if nm == "d":
            # dec wih shares S with whh; keep its 4-sigma inside e3m4 range
            while float(wi.std()) * S > F8_MAX / 4.2:
                S /= 2.0
        scales[nm] = S
        d[f"wih_{nm}"] = _pack_T(wi * S, FT, G, dt=FP8)
        d[f"whh_{nm}"] = _pack_T(wh * S, KT, G, dt=FP8)
    # lin/tp consume stored h'' = 2h
    lw = np.asarray(inputs["lin_w"], np.float32) * 0.5
    S_lin = _pow2(1.0 / max(float(lw.std()), 1e-12))
    scales["lin"] = S_lin
    d["linT"] = _pack_T(lw * S_lin, KT, F, dt=FP8)
    d["tpT"] = _pack_T(np.asarray(inputs["tp_w"], np.float32) * 0.5, KT, P)
    d["b_tf"] = np.asarray(inputs["tf_b"], np.float32)
    for nm, bi, bh in (("e", "enc_bih", "enc_bhh"),
                       ("p", "encp_bih", "encp_bhh"),
                       ("d", "dec_bih", "dec_bhh")):
        d[f"b_{nm}"] = ((np.asarray(inputs[bi], np.float32)
                         + np.asarray(inputs[bh], np.float32))[_GPERM]
                        * _GROWS * scales[nm])
    d["b_lin"] = np.asarray(inputs["lin_b"], np.float32) * S_lin
    d["b_tp"] = np.asarray(inputs["tp_b"], np.float32)
    d["scales"] = scales
    return d


def _bias_flags(w):
    return tuple(bool(np.any(w[k])) for k in
                 ("b_tf", "b_e", "b_p", "b_d", "b_lin", "b_tp"))


# ------------------------------------------------------------ device build

def build_model(key=((False,) * 6, 1.0, 1.0, 1.0, 1.0), loop_iters=1,
                ablate=(), warm_fillers=False, debug=False):
    bias_flags, S_e, S_p, S_d, S_lin = key
    gscales = {"e": 1.0 / S_e, "p": 1.0 / S_p, "d": 1.0 / S_d}
    has_btf, has_be, has_bp, has_bd, has_blin, has_btp = bias_flags
    any_bias = any(bias_flags)

    nc = bass.Bass()

    xT_d = nc.dram_tensor("xT", [P, T_IN * B], BF, kind="ExternalInput")
    zT_d = nc.dram_tensor("zT", [P, T_IN * B], BF, kind="ExternalInput")
    residT_d = nc.dram_tensor("residT", [P, T_OUT * B], FP32,
                              kind="ExternalInput")
    tfT_d = nc.dram_tensor("tfT", [P, F], BF, kind="ExternalInput")
    wih_d_d = {}
    whh_d_d = {}
    for nm in ("e", "p", "d"):
        wih_d_d[nm] = nc.dram_tensor(f"wih_{nm}", [128, FT * G], F8,
                                     kind="ExternalInput")
        whh_d_d[nm] = nc.dram_tensor(f"whh_{nm}", [128, KT * G], F8,
                                     kind="ExternalInput")
    linT_d = nc.dram_tensor("linT", [128, KT * F], F8, kind="ExternalInput")
    dmaskT_d = nc.dram_tensor("dmaskT", [B, TCAT * B], BF,
                              kind="ExternalInput")
    tpT_d = nc.dram_tensor("tpT", [128, KT * P], BF, kind="ExternalInput")
    bias_d = {}
    for key, flag, width in (("b_tf", has_btf, F), ("b_e", has_be, G),
                             ("b_p", has_bp, G), ("b_d", has_bd, G),
                             ("b_lin", has_blin, F), ("b_tp", has_btp, P)):
        if flag:
            bias_d[key] = nc.dram_tensor(key, [1, width], BF,
                                         kind="ExternalInput")
    out_d = nc.dram_tensor("oT", [P, T_OUT * B], FP32, kind="ExternalOutput")
    dbg_d = {}
    if debug:
        for nm, shp, dt in (("dbg_xf", [128, FT * T_IN * B], BF),
                            ("dbg_cat", [128, KT * B * TCAT], BF),
                            ("dbg_catT", [128, NCH * H], BF),
                            ("dbg_c", [128, KT * B], FP32),
                            ("dbg_s0", [B, TCAT], FP32),
                            ("dbg_e0", [B, TCAT], BF),
                            ("dbg_aw0", [B, B * TCAT], BF),
                            ("dbg_atth0", [128, KT * B], BF),
                            ("dbg_h1", [128, KT * B], BF),
                            ("dbg_c1", [128, KT * B], FP32),
                            ("dbg_inp0", [128, FT * B], BF),
                            ("dbg_g0", [128, MT * B], FP32),
                            ("dbg_tall0", [128, 4 * KT * B], FP32)):
            dbg_d[nm] = nc.dram_tensor(nm, shp, dt, kind="ExternalOutput")

    with tile.TileContext(nc) as tc:
        with (
            tc.tile_pool(name="singles", bufs=1) as singles,
            tc.tile_pool(name="wih_pool", bufs=2) as wih_pool,
            tc.tile_pool(name="whh_pool", bufs=2) as whh_pool,
            tc.tile_pool(name="gx_pool", bufs=2) as gx_pool,
            tc.tile_pool(name="pG_pool", bufs=2, space="PSUM") as pG_pool,
            tc.tile_pool(name="pShare", bufs=1, space="PSUM") as pShare,
            tc.tile_pool(name="pSmall", bufs=2, space="PSUM") as pSmall,
        ):
            def body(_it=None):
                # ------------- constant/static loads --------------------
                # order/queues chosen so ToFeature inputs land first, then
                # the encoder weights; late-use tensors trail on gpsimd
                tfT = singles.tile([P, F], BF, tag="tfT")
                nc.sync.dma_start(out=tfT, in_=tfT_d[:, :])
                xT = singles.tile([P, T_IN * B], BF, tag="xT")
                nc.sync.dma_start(out=xT, in_=xT_d[:, :])
                zT = singles.tile([P, T_IN * B], BF, tag="zT")
                nc.sync.dma_start(out=zT, in_=zT_d[:, :])
                residT = singles.tile([P, T_OUT * B], FP32, tag="residT")
                nc.sync.dma_start(out=residT, in_=residT_d[:, :])
                linT = singles.tile([128, KT * F], F8, tag="linT")
                nc.sync.dma_start(out=linT, in_=linT_d[:, :])
                tpT = singles.tile([128, KT * P], BF, tag="tpT")
                nc.sync.dma_start(out=tpT, in_=tpT_d[:, :])
                dmaskT = singles.tile([B, TCAT * B], BF, tag="dmaskT")
                nc.sync.dma_start(out=dmaskT, in_=dmaskT_d[:, :])

                bias_sb = {}
                for key, dram in bias_d.items():
                    t = singles.tile(list(dram.shape), BF, tag=key)
                    nc.sync.dma_start(out=t, in_=dram[:, :])
                    bias_sb[key] = t

                ident = singles.tile([128, 128], BF, tag="ident")
                from concourse.masks import make_identity
                make_identity(nc, ident)
                if any_bias:
                    ones_n = singles.tile([1, T_IN * B], BF, tag="ones_n")
                    nc.vector.memset(ones_n, 1.0)

                cat = singles.tile([128, KT, TCAT, B], BF, tag="cat")
                # catT: 6 chunks; chunk j rows r = b*4 + tl represent
                # slot 4j+tl, batch b (chunk 5: rows = batch, slot 20)
                catT = singles.tile([128, NCH, H], BF, tag="catT")
                if debug:
                    nc.vector.memset(catT, 0.0)
                c_e = singles.tile([128, KT, B, 1], FP32, tag="c_e")
                c_p = singles.tile([128, KT, B, 1], FP32, tag="c_p")

                c_bf = singles.tile([128, KT, B, 1], BF, tag="c_bf")
                atth = singles.tile([128, KT, B], BF, tag="atth")
                inp_bf = singles.tile([128, FT, B], BF, tag="inp_bf")
                masked32 = singles.tile([B, B * TCAT], FP32, tag="masked32")
                scoresbt = singles.tile([B, TCAT], FP32, tag="scoresbt")
                neg_mx = singles.tile([B, 1], FP32, tag="neg_mx")
                e_bf = singles.tile([B, TCAT], BF, tag="e_bf")
                e_nrm = singles.tile([B, TCAT], BF, tag="e_nrm")
                ssum32 = singles.tile([B, 1], FP32, tag="ssum32")
                rs32 = singles.tile([B, 1], FP32, tag="rs32")
                aw_m = singles.tile([B, B * TCAT], BF, tag="aw_m")
                A_sb = singles.tile([128, NCH, B], BF, tag="A_sb")
                t_all = singles.tile([128, 4 * KT, B], FP32, tag="t_all")
                th_t = singles.tile([128, KT, B], FP32, tag="th_t")
                u_t = singles.tile([128, KT, B], FP32, tag="u_t")
                v_t = singles.tile([128, KT, B], FP32, tag="v_t")
                oT_sb = singles.tile([P, T_OUT * B], FP32, tag="oT_sb")

                xf = singles.tile([128, FT, T_IN * B], BF, tag="xf")
                zf = singles.tile([128, FT, T_IN * B], BF, tag="zf")

                def pe_filler(dep_ap):
                    # Tiny matmul dependent on a just-produced DVE/ACT tile:
                    # keeps the PE p-state warm across long DVE/ACT chains.
                    if not warm_fillers:
                        return
                    fps = pSmall.tile([1, 8], FP32, tag="sm")
                    nc.tensor.matmul(fps[:, 0:1], dep_ap, dep_ap,
                                     start=True, stop=True)

                # ------------- ToFeature --------------------------------
                def to_feature(src, dst):
                    for ft in range(FT):
                        ps = pG_pool.tile([128, T_IN * B], FP32, tag="pG")
                        nc.tensor.matmul(ps, tfT[:, ft * 128:(ft + 1) * 128],
                                         src[:, :], start=True,
                                         stop=not has_btf)
                        if has_btf:
                            nc.tensor.matmul(
                                ps,
                                bias_sb["b_tf"][0:1, ft * 128:(ft + 1) * 128],
                                ones_n[0:1, :], start=False, stop=True)
                        nc.vector.tensor_copy(out=dst[:, ft, :], in_=ps)

                to_feature(xT, xf)
                to_feature(zT, zf)
                if debug:
                    nc.sync.dma_start(
                        out=dbg_d["dbg_xf"][:, :],
                        in_=xf.rearrange("p a b -> p (a b)"))

                # ------------- encoder gates_x precompute ----------------
                def gates_x(wih_sb, src, dst, bkey, mts=None):
                    for mt in (range(MT) if mts is None else mts):
                        ps = pG_pool.tile([128, T_IN * B], FP32, tag="pG")
                        for kt in range(FT):
                            nc.tensor.matmul(
                                ps,
                                wih_sb[:, kt * G + mt * 128:
                                       kt * G + (mt + 1) * 128],
                                src[:, kt, :],
                                start=(kt == 0),
                                stop=(kt == FT - 1 and bkey is None),
                            )
                        if bkey is not None:
                            nc.tensor.matmul(
                                ps, bias_sb[bkey][0:1, mt * 128:(mt + 1) * 128],
                                ones_n[0:1, :], start=False, stop=True)
                        if mt % 2 == 0:
                            nc.vector.tensor_copy(
                                out=dst[:, :, mt, :],
                                in_=ps.rearrange("p (t b) -> p t b", b=B))
                        else:
                            nc.scalar.copy(
                                out=dst[:, :, mt, :],
                                in_=ps.rearrange("p (t b) -> p t b", b=B))

                wih_e = wih_pool.tile([128, FT * G], F8, tag="wih")
                for kt in range(FT):
                    nc.sync.dma_start(out=wih_e[:, kt * G:(kt + 1) * G],
                                      in_=wih_d_d["e"][:, kt * G:(kt + 1) * G])
                gx_e = gx_pool.tile([128, T_IN, MT, B], BF, tag="gx")
                gates_x(wih_e, xf, gx_e, "b_e" if has_be else None)


                # ------------- LSTM gate tail (tanh-only form) ----------
                # gates packed [i, g, f, o]; t = tanh(gates) (i,f,o at
                # half-arg via host scaling), split in two ACT ops so the
                # DVE chain starts after the first half:
                # u = (t_i+1)*t_g ; v = (t_f+1)*c'' ; c''_new = v/2 + u
                # th = tanh(c''/2) ; h'' = (t_o+1)*th
                def lstm_tail(gsrc, c_tile, h_out, first_step,
                              emit_cbf=False, gscale=1.0):
                    nc.scalar.activation(out=t_all[:, 0:2 * KT, :],
                                         in_=gsrc[:, 0:2 * KT, :],
                                         func=ACT_TANH, scale=gscale)
                    nc.scalar.activation(out=t_all[:, 2 * KT:4 * KT, :],
                                         in_=gsrc[:, 2 * KT:4 * KT, :],
                                         func=ACT_TANH, scale=gscale)
                    cs = c_tile[:, :, :, 0]
                    pe_filler(t_all[0:1, 0:1, 0:1])
                    ti = t_all[:, 0:KT, :]
                    tg = t_all[:, KT:2 * KT, :]
                    tf_ = t_all[:, 2 * KT:3 * KT, :]
                    to = t_all[:, 3 * KT:4 * KT, :]
                    if first_step:
                        nc.vector.scalar_tensor_tensor(
                            out=cs, in0=ti, scalar=1.0, in1=tg,
                            op0=ALU_ADD, op1=ALU_MUL)
                    else:
                        nc.vector.scalar_tensor_tensor(
                            out=u_t, in0=ti, scalar=1.0, in1=tg,
                            op0=ALU_ADD, op1=ALU_MUL)
                        nc.vector.scalar_tensor_tensor(
                            out=v_t, in0=tf_, scalar=1.0, in1=cs,
                            op0=ALU_ADD, op1=ALU_MUL)
                        nc.vector.scalar_tensor_tensor(
                            out=cs, in0=v_t, scalar=0.5, in1=u_t,
                            op0=ALU_MUL, op1=ALU_ADD)
                    if emit_cbf:
                        # scores want true c = c''/2, against cat'' = 2h:
                        # c_bf = c''/4
                        nc.vector.tensor_scalar_mul(c_bf[:, :, :, 0], cs, 0.25)
                    # th and h split by kt-halves: kt 0:4 consumers of the
                    # new h (whh/scoresB/catT/lin kt loops) start one half-op
                    # earlier; the full-width ops would gate them on all of h
                    HK = KT // 2
                    nc.scalar.activation(out=th_t[:, 0:HK, :],
                                         in_=cs[:, 0:HK, :], func=ACT_TANH,
                                         scale=0.5)
                    pe_filler(th_t[0:1, 0:1, 0:1])
                    nc.vector.scalar_tensor_tensor(
                        out=h_out[:, 0:HK, :], in0=to[:, 0:HK, :], scalar=1.0,
                        in1=th_t[:, 0:HK, :],
                        op0=ALU_ADD, op1=ALU_MUL)
                    nc.scalar.activation(out=th_t[:, HK:KT, :],
                                         in_=cs[:, HK:KT, :], func=ACT_TANH,
                                         scale=0.5)
                    nc.vector.scalar_tensor_tensor(
                        out=h_out[:, HK:KT, :], in0=to[:, HK:KT, :],
                        scalar=1.0, in1=th_t[:, HK:KT, :],
                        op0=ALU_ADD, op1=ALU_MUL)

                # ------------- catT chunk maintenance --------------------
                # chunk j rows r = tl*32 + b represent slot 4j+tl, batch b
                chunk_state = {}

                def catT_slot(slot):
                    j, tl = divmod(slot, 4)
                    if tl == 0:
                        chunk_state[j] = pShare.tile([128, H], BF, tag="pA",
                                                     name=f"tch{j}")
                    tps = chunk_state[j]
                    for kt in range(KT):
                        nc.tensor.matmul(
                            tps[tl * 32:(tl + 1) * 32,
                                kt * 128:(kt + 1) * 128],
                            cat[:, kt, slot, :],
                            ident, is_transpose=True, start=True, stop=True,
                            tile_position=(0, tl * 32),
                            skip_group_check=(tl > 0))
                    if tl == 3:
                        nc.scalar.copy(out=catT[:, j, :], in_=tps)
                        del chunk_state[j]

                def catT_dec_slot():
                    # decoder slot 20 -> catT chunk 5 rows 0..31; copy-back
                    # split across ACT and DVE
                    tps = pSmall.tile([B, H], BF, tag="sm")
                    for kt in range(KT):
                        nc.tensor.matmul(
                            tps[:, kt * 128:(kt + 1) * 128],
                            cat[:, kt, SLOT_DEC, :],
                            ident, is_transpose=True, start=True, stop=True)
                    nc.scalar.copy(out=catT[0:B, NCH - 1, 0:512],
                                   in_=tps[:, 0:512])
                    nc.vector.tensor_copy(out=catT[0:B, NCH - 1, 512:H],
                                          in_=tps[:, 512:H])

                # ------------- encoder chains ----------------------------
                def run_encoder(whh_sb, gx, c_tile, slot0, gscale,
                                fill=None):
                    for t in range(T_IN):
                        slot = slot0 + t
                        h_out = cat[:, :, slot, :]
                        if t == 0:
                            lstm_tail(gx[:, 0, :, :], c_tile, h_out, True,
                                      gscale=gscale)
                            if fill is not None:
                                fill(t)
                        else:
                            gps = pG_pool.tile([128, MT, B], FP32, tag="pG")
                            # seed with gx[t] via two N=512 identity folds
                            for hh in range(2):
                                nc.tensor.matmul(
                                    gps[:, hh * 16:(hh + 1) * 16, :].rearrange(
                                        "p m b -> p (m b)"),
                                    ident,
                                    gx[:, t, hh * 16:(hh + 1) * 16, :]
                                    .rearrange("p m b -> p (m b)"),
                                    start=True, stop=False)
                            prev = cat[:, :, slot - 1, :]
                            for kt in range(KT):
                                for mt in range(MT):
                                    nc.tensor.matmul(
                                        gps[:, mt, :],
                                        whh_sb[:, kt * G + mt * 128:
                                               kt * G + (mt + 1) * 128],
                                        prev[:, kt, :],
                                        start=False,
                                        stop=(kt == KT - 1
                                              and mt % 16 == 15),
                                    )
                            if fill is not None:
                                fill(t)
                            lstm_tail(gps, c_tile, h_out, False,
                                      gscale=gscale)
                        catT_slot(slot)

                whh_e = whh_pool.tile([128, KT * G], F8, tag="whh")
                for kt in range(KT):
                    nc.sync.dma_start(out=whh_e[:, kt * G:(kt + 1) * G],
                                      in_=whh_d_d["e"][:, kt * G:(kt + 1) * G])
                wih_p = wih_pool.tile([128, FT * G], F8, tag="wih")
                for kt in range(FT):
                    nc.sync.dma_start(out=wih_p[:, kt * G:(kt + 1) * G],
                                      in_=wih_d_d["p"][:, kt * G:(kt + 1) * G])
                gx_p = gx_pool.tile([128, T_IN, MT, B], BF, tag="gx")
                if "enc" not in ablate:
                    bkey_p = "b_p" if has_bp else None

                    def _fill_e(t):
                        gates_x(wih_p, zf, gx_p, bkey_p,
                                mts=(2 * t, 2 * t + 1))

                    run_encoder(whh_e, gx_e, c_e, 0, gscales["e"],
                                fill=_fill_e)
                    gates_x(wih_p, zf, gx_p, bkey_p,
                            mts=range(2 * T_IN, MT))
                else:
                    gates_x(wih_p, zf, gx_p, "b_p" if has_bp else None)
                    nc.vector.memset(cat, 0.01)
                    nc.vector.memset(catT, 0.01)
                    nc.vector.memset(c_e, 0.01)
                    nc.vector.memset(c_p, 0.01)

                whh_p = whh_pool.tile([128, KT * G], F8, tag="whh")
                for kt in range(KT):
                    nc.sync.dma_start(out=whh_p[:, kt * G:(kt + 1) * G],
                                      in_=whh_d_d["p"][:, kt * G:(kt + 1) * G])
                if "enc" not in ablate:
                    run_encoder(whh_p, gx_p, c_p, T_IN, gscales["p"])

                # ------------- decoder ----------------------------------
                wih_dd = wih_pool.tile([128, FT * G], F8, tag="wih")
                for kt in range(FT):
                    nc.sync.dma_start(out=wih_dd[:, kt * G:(kt + 1) * G],
                                      in_=wih_d_d["d"][:, kt * G:(kt + 1) * G])
                whh_dd = whh_pool.tile([128, KT * G], F8, tag="whh")
                for kt in range(KT):
                    nc.sync.dma_start(out=whh_dd[:, kt * G:(kt + 1) * G],
                                      in_=whh_d_d["d"][:, kt * G:(kt + 1) * G])

                nc.vector.tensor_copy(out=cat[:, :, SLOT_DEC, :],
                                      in_=cat[:, :, T_IN - 1, :])
                c_d = c_e
                nc.vector.tensor_scalar_mul(c_bf[:, :, :, 0],
                                            c_d[:, :, :, 0], 0.25)

                def topose_mm(ti_):
                    # out column block for one decoder step; runs in the PE
                    # bubble while the current step's DVE/ACT tail executes.
                    # Reads the h still sitting in the decoder cat slot.
                    tps_o = pSmall.tile([P, B], FP32, tag="sm")
                    for kt in range(KT):
                        nc.tensor.matmul(
                            tps_o,
                            tpT[:, kt * P:(kt + 1) * P],
                            cat[:, kt, SLOT_DEC, :],
                            start=(kt == 0),
                            stop=(kt == KT - 1 and not has_btp))
                    if has_btp:
                        nc.tensor.matmul(
                            tps_o, bias_sb["b_tp"][0:1, :],
                            ones_n[0:1, 0:B], start=False, stop=True)
                    return tps_o

                def topose_add(ti_, tps_o):
                    nc.vector.tensor_add(
                        oT_sb[:, ti_ * B:(ti_ + 1) * B],
                        tps_o,
                        residT[:, ti_ * B:(ti_ + 1) * B])
                if "dec" not in ablate:
                    catT_dec_slot()

                if debug:
                    nc.sync.dma_start(
                        out=dbg_d["dbg_cat"][:, :],
                        in_=cat.rearrange("p a b c -> p (a b c)"))
                    nc.sync.dma_start(
                        out=dbg_d["dbg_catT"][:, :],
                        in_=catT.rearrange("p a b -> p (a b)"))
                    nc.sync.dma_start(
                        out=dbg_d["dbg_c"][:, :],
                        in_=c_d.rearrange("p a b c -> p (a b c)"))

                nhalf = B // 2

                dec_steps = 0 if "dec" in ablate else T_OUT
                for t in range(dec_steps):
                    # --- scores: diagonal matmul, c_bf stationary;
                    # t-major contiguous moving chunks (16 slots | 5 slots).
                    # All chunk-A MMs first: they read only static encoder
                    # slots + c_bf (ready mid-tail), while chunk B includes
                    # slot 20 (the fresh h, ready only at tail end) -- the
                    # FIFO PE would otherwise stall chunk A behind B ---
                    scd = pShare.tile([B, 2, 512], FP32, tag="pA")
                    for kt in range(KT):
                        nc.tensor.matmul(
                            scd[:, 0, 0:16 * B],
                            c_bf[:, kt, :, 0],
                            cat[:, kt, 0:16, :].rearrange(
                                "p t b -> p (t b)"),
                            start=(kt == 0), stop=(kt == KT - 1))
                    for kt in range(KT):
                        nc.tensor.matmul(
                            scd[:, 1, 0:(TCAT - 16) * B],
                            c_bf[:, kt, :, 0],
                            cat[:, kt, 16:TCAT, :].rearrange(
                                "p t b -> p (t b)"),
                            start=(kt == 0), stop=(kt == KT - 1))

                    # --- lin: inp = h'' @ (lin_w/2).T  (on PE) ---
                    ips = pSmall.tile([128, FT, B], FP32, tag="sm")
                    for kt in range(KT):
                        for mt in range(FT):
                            nc.tensor.matmul(
                                ips[:, mt, :],
                                linT[:, kt * F + mt * 128:
                                     kt * F + (mt + 1) * 128],
                                cat[:, kt, SLOT_DEC, :],
                                start=(kt == 0 and mt == 0),
                                stop=(kt == KT - 1 and not has_blin
                                      and mt == FT - 1),
                            )
                    if has_blin:
                        for mt in range(FT):
                            nc.tensor.matmul(
                                ips[:, mt, :],
                                bias_sb["b_lin"][0:1, mt * 128:(mt + 1) * 128],
                                ones_n[0:1, 0:B], start=False,
                                stop=(mt == FT - 1))
                    nc.scalar.activation(out=inp_bf, in_=ips,
                                         func=ACT_COPY,
                                         scale=1.0 / S_lin)
                    # catT chunk 5 (prev h transposed; deps long ready):
                    # emitted here so its PE work covers the inp staging
                    # edge between lin and wih
                    if t > 0:
                        catT_dec_slot()
                    if debug and t == 0:
                        nc.sync.dma_start(
                            out=dbg_d["dbg_inp0"][:, :],
                            in_=inp_bf.rearrange("p a b -> p (a b)"))

                    # --- wih gates seed the PSUM accumulator early ---
                    gps = pG_pool.tile([128, MT, B], FP32, tag="pG")
                    # start only on the first write of each PSUM bank: the
                    # bank-wide pending-zero makes every later first-touch
                    # store, and accumulation chains stay intact
                    for kt in range(FT):
                        for mt in range(MT):
                            nc.tensor.matmul(
                                gps[:, mt, :],
                                wih_dd[:, kt * G + mt * 128:
                                       kt * G + (mt + 1) * 128],
                                inp_bf[:, kt, :],
                                start=(kt == 0 and mt % 16 == 0),
                                stop=False)
                    if has_bd:
                        for mt in range(MT):
                            nc.tensor.matmul(
                                gps[:, mt, :],
                                bias_sb["b_d"][0:1, mt * 128:(mt + 1) * 128],
                                ones_n[0:1, 0:B], start=False, stop=False)

                    # --- softmax over 21 slots (batch on partitions) ---
                    nc.vector.tensor_mul(
                        masked32[:, 0:16 * B],
                        scd[:, 0, 0:16 * B],
                        dmaskT[:, 0:16 * B])
                    nc.vector.tensor_mul(
                        masked32[:, 16 * B:TCAT * B],
                        scd[:, 1, 0:(TCAT - 16) * B],
                        dmaskT[:, 16 * B:TCAT * B])
                    nc.vector.tensor_reduce(
                        scoresbt,
                        masked32.rearrange("p (t b) -> p t b", t=TCAT),
                        axis=mybir.AxisListType.X, op=mybir.AluOpType.add)
                    pe_filler(scoresbt[:, 0:1])
                    nc.vector.tensor_reduce(
                        neg_mx, scoresbt, axis=mybir.AxisListType.X,
                        op=mybir.AluOpType.max, negate=True)
                    nc.scalar.activation(
                        out=e_bf, in_=scoresbt, func=ACT_EXP,
                        bias=neg_mx, accum_out=ssum32)
                    nc.vector.reciprocal(rs32, ssum32)
                    # normalized weights: 1/sum folded here so the direct
                    # (h,b)-layout context matmuls need no output scaling
                    nc.vector.tensor_mul(
                        e_nrm, e_bf, rs32.to_broadcast((B, TCAT)))
                    nc.vector.tensor_mul(
                        aw_m.rearrange("p (t b) -> p t b", t=TCAT),
                        dmaskT.rearrange("p (t b) -> p t b", t=TCAT),
                        e_nrm.unsqueeze(2).to_broadcast((B, TCAT, B)))
                    if debug and t == 0:
                        nc.sync.dma_start(out=dbg_d["dbg_s0"][:, :],
                                          in_=scoresbt)
                        nc.sync.dma_start(out=dbg_d["dbg_e0"][:, :], in_=e_bf)
                        nc.sync.dma_start(out=dbg_d["dbg_aw0"][:, :], in_=aw_m)

                    # --- context via PE: A = aw_m^T in 6 chunk
                    #     transposes, then ctxT(b,h) = sum_j A_j^T @ catT_j ---
                    pA = pSmall.tile([128, NCH, B], BF, tag="sm")
                    for j in range(NCH):
                        cols = 128 if j < NCH - 1 else B
                        nc.tensor.matmul(
                            pA[0:cols, j, :],
                            aw_m[:, j * 128:j * 128 + cols],
                            ident[0:B, 0:B],
                            is_transpose=True, start=True, stop=True)
                    nc.scalar.copy(out=A_sb[:, 0:NCH - 1, :],
                                   in_=pA[:, 0:NCH - 1, :])
                    nc.vector.tensor_copy(out=A_sb[0:B, NCH - 1, :],
                                          in_=pA[0:B, NCH - 1, :])
                    tp_ps = topose_mm(t - 1) if t > 0 else None

                    # context computed directly in (h, b) layout:
                    # atth(h,b) = sum_j catT_j[:, hslice].T @ A_j ; per
                    # h-half copies (ACT then DVE) so whh chases half 0
                    atps = pSmall.tile([128, KT, B], FP32, tag="sm")
                    for hh in range(2):
                        for kt in range(4 * hh, 4 * hh + 4):
                            for j in range(NCH):
                                rows = 128 if j < NCH - 1 else B
                                nc.tensor.matmul(
                                    atps[:, kt, :],
                                    catT[0:rows, j,
                                         kt * 128:(kt + 1) * 128],
                                    A_sb[0:rows, j, :],
                                    start=(j == 0), stop=(j == NCH - 1))
                        if hh == 0:
                            nc.scalar.copy(
                                out=atth[:, 0:4, :], in_=atps[:, 0:4, :])
                        else:
                            nc.vector.tensor_copy(
                                out=atth[:, 4:8, :], in_=atps[:, 4:8, :])
                    if debug and t == 0:
                        nc.sync.dma_start(
                            out=dbg_d["dbg_atth0"][:, :],
                            in_=atth.rearrange("p a b -> p (a b)"))

                    # --- whh gates accumulate onto the wih seed ---
                    for hf in range(2):
                        for kt in range(4 * hf, 4 * hf + 4):
                            for mt in range(MT):
                                nc.tensor.matmul(
                                    gps[:, mt, :],
                                    whh_dd[:, kt * G + mt * 128:
                                           kt * G + (mt + 1) * 128],
                                    atth[:, kt, :],
                                    start=False,
                                    stop=(kt == KT - 1 and mt % 16 == 15))

                    if debug and t == 0:
                        dbg_g = gx_pool.tile([128, MT, B], FP32, tag="dbgg",
                                             bufs=1)
                        nc.vector.tensor_copy(out=dbg_g, in_=gps)
                        nc.sync.dma_start(
                            out=dbg_d["dbg_g0"][:, :],
                            in_=dbg_g.rearrange("p a b -> p (a b)"))
                    lstm_tail(gps, c_d, cat[:, :, SLOT_DEC, :], False,
                              emit_cbf=(t < T_OUT - 1),
                              gscale=gscales["d"])
                    if debug and t == 0:
                        nc.sync.dma_start(
                            out=dbg_d["dbg_tall0"][:, :],
                            in_=t_all.rearrange("p a b -> p (a b)"))
                    if tp_ps is not None:
                        topose_add(t - 1, tp_ps)
                    if debug and t == 0:
                        dbgh1 = gx_pool.tile([128, KT, B], BF, tag="dbgh1",
                                             bufs=1)
                        nc.vector.tensor_copy(out=dbgh1,
                                              in_=cat[:, :, SLOT_DEC, :])
                        nc.sync.dma_start(
                            out=dbg_d["dbg_h1"][:, :],
                            in_=dbgh1.rearrange("p a b -> p (a b)"))
                        nc.sync.dma_start(
                            out=dbg_d["dbg_c1"][:, :],
                            in_=c_d.rearrange("p a b c -> p (a b c)"))
                # ------------- ToPose tail + output ---------------------
                if "dec" in ablate:
                    return
                topose_add(T_OUT - 1, topose_mm(T_OUT - 1))
                nc.sync.dma_start(out=out_d[:, :], in_=oT_sb)

            if loop_iters > 1:
                with tc.For_i(0, loop_iters, 1, name="rep"):
                    body()
            else:
                body()

    return nc


# ------------------------------------------------------------- entry point

_model_cache = {}


def _get_model(key):
    if key not in _model_cache:
        _model_cache[key] = build_model(key)
    return _model_cache[key]


def make_in_maps(inputs):
    """Host-side packing: returns per-core input maps and the model key."""
    w = _prep_weights(inputs)
    sc = w.pop("scales")
    flags = _bias_flags(w)
    key = (flags, sc["e"], sc["p"], sc["d"], sc["lin"])
    x = np.asarray(inputs["x"], dtype=np.float32)
    z = np.asarray(inputs["z"], dtype=np.float32)
    fr = np.asarray(inputs["for_resid"], dtype=np.float32)

    dmask = np.zeros((B, B, TCAT), dtype=np.float32)
    for b in range(B):
        dmask[b, b, :] = 1.0
    shared = {
        "tfT": w["tfT"], "linT": w["linT"], "tpT": w["tpT"],
        "dmaskT": np.ascontiguousarray(
            dmask.transpose(0, 2, 1).reshape(B, TCAT * B)).astype(BF16),
    }
    for nm in ("e", "p", "d"):
        shared[f"wih_{nm}"] = w[f"wih_{nm}"]
        shared[f"whh_{nm}"] = w[f"whh_{nm}"]
    names = ("b_tf", "b_e", "b_p", "b_d", "b_lin", "b_tp")
    for f, name in zip(flags, names):
        if f:
            shared[name] = np.ascontiguousarray(
                w[name][None, :]).astype(BF16)

    in_maps = []
    for c in range(N_CORES):
        sl = slice(c * B, (c + 1) * B)
        m = dict(shared)
        m["xT"] = np.ascontiguousarray(
            x[sl].transpose(2, 1, 0).reshape(P, T_IN * B)).astype(BF16)
        m["zT"] = np.ascontiguousarray(
            z[sl].transpose(2, 1, 0).reshape(P, T_IN * B)).astype(BF16)
        m["residT"] = np.ascontiguousarray(
            fr[sl].transpose(2, 1, 0).reshape(P, T_OUT * B))
        in_maps.append(m)
    return in_maps, key


def unshard_output(results):
    outs = []
    for c in range(N_CORES):
        oT = np.asarray(results[c]["oT"])  # (66, 800)
        outs.append(oT.reshape(P, T_OUT, B).transpose(2, 1, 0))
    return np.ascontiguousarray(np.concatenate(outs, axis=0),
                                dtype=np.float32)


def kernel(**inputs) -> np.ndarray:
    in_maps, key = make_in_maps(inputs)
    nc = _get_model(key)
    res = run_bass_kernel_spmd(nc, in_maps, core_ids=list(range(N_CORES)))
    return unshard_output(res.results)

